# revision 18
# baseline (speedup 1.0000x reference)
"""Trainium2 Bass kernel for a transformer decoder layer (self-attn +
cross-attn + FFN), fp8-E4M3 DoubleRow edition.

Sharding: 8 cores = 4 batches x 2 halves; core h of a batch owns the
interleaved query tiles {h, h+2, ..., h+14} (causal load balance) and
computes the FULL K/V for its batch locally (no collectives -- the cost
model's 120us/AllGather dwarfs the ~27us of duplicated fp8 projection
work per stage).

All matmuls run fp8 E4M3 with perf_mode=DoubleRow (256-deep contraction
per instruction, 0.5 cycles/row). Accuracy is recovered with same-scale
residual compensation: for x@W the kernel accumulates x8@W8 + x8@dW8
(+ dx8@W8) in one PSUM group, where dW8 = q8(W*s - W8) etc. E4M3
represents the small residuals near-exactly at the same scale, so each
compensated operand contributes ~0.1% error instead of ~3.6%.

Scales (powers of 2, folded into ACT drain scales / exp bias):
  activations (y, Z, y1, y2): x16     weights: x1024
  Q, K, V on-chip: x32 (drain 2^-9)   E = 8*exp(score) via exp bias ln 8
  h (relu out): x16 (drain 2^-10)     denominator ones-vector = 32.0
Causal masking adds -1e6 to masked score psum before the exp drain.
"""

from contextlib import ExitStack

import numpy as np
import ml_dtypes

import concourse.bass as bass
import concourse.mybir as mybir
import concourse.tile as tile
from concourse import bacc
from concourse.bass_utils import run_bass_kernel_spmd
from concourse.masks import make_identity

f32 = mybir.dt.float32
f16 = mybir.dt.float16
f8 = mybir.dt.float8e4
E4 = ml_dtypes.float8_e4m3
DR = mybir.MatmulPerfMode.DoubleRow

P = 128
D = 1024          # d_model
S = 2048          # kv sequence length
NQ = 1024         # query tokens per core
DFF = 4096
DTI = D // P      # 8 d-model partition tiles
DPI = DTI // 2    # 4 d-model pair tiles
KTI = S // P      # 16 kv token tiles
KPI = KTI // 2    # 8 kv pair tiles
QTI = NQ // P     # 8 query tiles
FTI = DFF // P    # 32 d_ff tiles
FPI = FTI // 2    # 16 d_ff pair tiles
NCH = NQ // 512   # 2 query chunks of 512
ACT = mybir.ActivationFunctionType
ALU = mybir.AluOpType
N_CORES = 8
LN8 = float(np.log(8.0))
# x-side compensation knobs (weights + V-drain comp always on)
XC_Y1 = False     # dyq/dykv terms in self-attn projections
XC_Z = False      # dz terms in K2/V2 (softmax averaging washes these out)
XC_Y1T = False    # dy1T terms in Q2 projection
XC_Y2T = True     # dy2T terms in FFN1
DS_QKV = 2.0 ** -9    # psum(16*1024) -> x32
DS_EXP = 2.0 ** -15   # psum(32*32) -> exp(score/32)
DS_H = 2.0 ** -10     # psum(16*1024) -> x16
DS_F = 2.0 ** -14     # psum(16*1024) -> true


def _self_visible(t, c):
    """Queries interleaved: core h owns global q-tiles {h, h+2, ...}; local
    q-tile u is global 2u+h <= 2u+1; chunk c (tiles 4c..4c+3) sees k-tile t
    iff t <= 2(4c+3)+1, i.e. t < 8(c+1)."""
    return t < 8 * (c + 1)


def _self_needs_mask(t, c):
    return t >= 8 * c


def build_nc(gb_trivial=True, bf1_zero=True):
    nc = bacc.Bacc("TRN2", target_bir_lowering=False, debug=False,
                   num_devices=N_CORES)

    def dp(name, shape, dt, out=False):
        return nc.declare_dram_parameter(name, shape, dt, isOutput=out)

    yqT_d = dp("yqT", [D, NQ], f8)
    dyqT_d = dp("dyqT", [D, NQ], f8)
    ykvT_d = dp("ykvT", [D, S], f8)
    dykvT_d = dp("dykvT", [D, S], f8)
    zT_d = dp("zT", [D, S], f8)
    dzT_d = dp("dzT", [D, S], f8)
    yres_d = dp("yres", [NQ, D], f16)
    qg_d = dp("qg", [NQ], f32)
    kg_d = dp("kg", [S], f32)
    ones8_d = dp("ones8", [2 * P, 16], f8)
    w_d = {}
    for n in ["wq1", "wk1", "wv1", "wq2", "wk2", "wv2"]:
        w_d[n] = dp(n, [D, D], f8)
        w_d["d" + n] = dp("d" + n, [D, D], f8)
    wf1_d = dp("wf1", [D, DFF], f8)
    dwf1_d = dp("dwf1", [D, DFF], f8)
    wf2_d = dp("wf2", [DFF, D], f8)
    dwf2_d = dp("dwf2", [DFF, D], f8)
    bf1_d = dp("bf1", [P, FTI], f32)
    vec_d = {n: dp(n, [D], f32)
             for n in ["bf2", "g1", "be1", "g2", "be2", "g3", "be3"]}
    out_d = dp("out", [NQ, D], f32, out=True)

    def bc(ap):  # broadcast a [n] dram vector across 128 partitions
        return bass.AP(tensor=ap.tensor, offset=ap.offset,
                       ap=[[0, P]] + [list(x) for x in ap.ap])

    def pear(dram, ncols):  # [R, ncols] dram -> [P, R//P, ncols]
        return dram.ap().rearrange("(t p) n -> p t n", p=P)

    with tile.TileContext(nc) as tc, ExitStack() as top:
        const = top.enter_context(tc.tile_pool(name="const", bufs=1))
        dramp = top.enter_context(tc.tile_pool(name="dramp", bufs=1,
                                               space="DRAM"))
        ident = const.tile([P, P], f16, name="ident", tag="ident")
        make_identity(nc, ident)
        kidx = const.tile([P, KTI], f32, name="kidx", tag="kidx")
        nc.sync.dma_start(out=kidx, in_=kg_d.ap().rearrange("(n p) -> p n", p=P))
        qgb = const.tile([P, NQ], f32, name="qgb", tag="qgb")
        nc.sync.dma_start(out=qgb, in_=bc(qg_d.ap()))
        eps = const.tile([P, 1], f32, name="eps", tag="eps")
        nc.vector.memset(eps, 1e-5)
        ln8b = const.tile([P, 1], f32, name="ln8b", tag="ln8b")
        nc.vector.memset(ln8b, LN8)
        bf1_sb = const.tile([P, FTI], f32, name="bf1_sb", tag="bf1")
        nc.sync.dma_start(out=bf1_sb, in_=bf1_d.ap())
        ones8 = const.tile([P, 2, 16], f8, name="ones8", tag="ones8")
        nc.sync.dma_start(
            out=ones8,
            in_=ones8_d.ap().rearrange("(two p) n -> p two n", p=P))

        def load_vec_bcast(pool, name):
            t = pool.tile([P, D], f32, name=f"{name}_sb", tag=f"vb_{name}")
            nc.sync.dma_start(out=t, in_=bc(vec_d[name].ap()))
            return t

        def load_pairs(pool, dram, wname, nrows, ncols, tag=None):
            t = pool.tile([P, nrows // P, ncols], f8, name=wname,
                          tag=tag or wname)
            nc.sync.dma_start(out=t, in_=pear(dram, ncols))
            return t

        def layer_norm(lnp, x, gb, bb, out):
            """out = (x - mean) * rsqrt(var + eps) [* gb + bb], per row."""
            stats = lnp.tile([P, 2, 6], f32, name="stats", tag="stats")
            nc.vector.bn_stats(out=stats[:, 0, :], in_=x[:, 0:512])
            nc.vector.bn_stats(out=stats[:, 1, :], in_=x[:, 512:1024])
            mv = lnp.tile([P, 2], f32, name="mv", tag="mv")
            nc.vector.bn_aggr(out=mv, in_=stats)
            std = lnp.tile([P, 1], f32, name="std", tag="std")
            nc.scalar.activation(out=std, in_=mv[:, 1:2], func=ACT.Sqrt,
                                 bias=eps, scale=1.0)
            rstd = lnp.tile([P, 1], f32, name="rstd", tag="rstd")
            nc.vector.reciprocal(rstd, std)
            if gb_trivial:
                nc.vector.tensor_scalar(out=out, in0=x, scalar1=mv[:, 0:1],
                                        scalar2=rstd, op0=ALU.subtract,
                                        op1=ALU.mult)
            else:
                tmp = lnp.tile([P, D], f32, name="lntmp", tag="lntmp", bufs=2)
                nc.vector.tensor_scalar(out=tmp, in0=x, scalar1=mv[:, 0:1],
                                        scalar2=rstd, op0=ALU.subtract,
                                        op1=ALU.mult)
                nc.vector.tensor_mul(out=tmp, in0=tmp, in1=gb)
                nc.vector.tensor_add(out=out, in0=tmp, in1=bb)

        def mm_terms(ps, terms, nt, csl, osl):
            """Accumulate sum of DoubleRow terms into psum ps.
            terms: list of (lhsT_tile, rhs_tile, lcols) -- lhsT sliced
            [:, 2t:2t+2, lcols], rhs sliced [:, 2t:2t+2, csl]."""
            last = len(terms) * nt - 1
            k = 0
            for (lt, rt, lsl) in terms:
                for t in range(nt):
                    nc.tensor.matmul(ps[:, osl] if osl else ps,
                                     lhsT=lt[:, 2 * t:2 * t + 2, lsl],
                                     rhs=rt[:, 2 * t:2 * t + 2, csl],
                                     start=(k == 0), stop=(k == last),
                                     perf_mode=DR)
                    k += 1

        def project_dT(psum, outp, out8, W8, dW8, X8, dX8, ncols, tag):
            """out8[:, i, cols] = (W.T @ X) * 2^-9; contraction d pairs.
            Compensation: + X@dW (+ dX@W if dX8 given). Two 512-col psum
            groups share a [P,1024] tile and drain with one ACT op."""
            terms0 = [(W8, X8), (dW8, X8)]
            if dX8 is not None:
                terms0.append((W8, dX8))
            for i in range(DTI):
                lsl = slice(i * P, (i + 1) * P)
                for ck in range(ncols // 1024):
                    ps = psum.tile([P, 1024], f32, name="ps_p",
                                   tag="ps_proj")
                    for hh in range(2):
                        csl = slice(ck * 1024 + hh * 512,
                                    ck * 1024 + (hh + 1) * 512)
                        mm_terms(ps[:, hh * 512:(hh + 1) * 512],
                                 [(a, b, lsl) for (a, b) in terms0],
                                 DPI, csl, None)
                    nc.scalar.activation(
                        out=out8[:, i, ck * 1024:(ck + 1) * 1024], in_=ps,
                        func=ACT.Copy, scale=DS_QKV)

        def project_v(psum, stgp, v8, dv8, W8, dW8, X8, dX8, tag):
            """v8[:, tk, :] = (X_tok^T @ W) * 2^-9 with residual dv8."""
            terms = [(X8, W8), (X8, dW8)]
            if dX8 is not None:
                terms.append((dX8, W8))
            for tk in range(KTI):
                lsl = slice(tk * P, (tk + 1) * P)
                ps = psum.tile([P, 1024], f32, name="ps_v", tag="ps_proj")
                for n in range(2):
                    csl = slice(n * 512, (n + 1) * 512)
                    last = len(terms) * DPI - 1
                    k = 0
                    for (lt, rt) in terms:
                        for t in range(DPI):
                            nc.tensor.matmul(ps[:, csl],
                                             lhsT=lt[:, 2 * t:2 * t + 2, lsl],
                                             rhs=rt[:, 2 * t:2 * t + 2, csl],
                                             start=(k == 0), stop=(k == last),
                                             perf_mode=DR)
                            k += 1
                v16 = stgp.tile([P, 1024], f16, name="v16", tag="v16")
                nc.scalar.activation(out=v16, in_=ps, func=ACT.Copy,
                                     scale=DS_QKV)
                nc.vector.tensor_copy(out=v8[:, tk, :], in_=v16)
                nc.vector.tensor_sub(out=dv8[:, tk, :], in0=v16,
                                     in1=v8[:, tk, :])

        def attention(stk, tagp, qT8, kT8, v8, dv8, resid, gb, bb, y_out,
                      masked):
            """scoresT = K^T Q per (k-tile, chunk) -> +maskbias -> exp drain
            to fp8 -> out = E^T (V + dV); denominators via ones32 matmul."""
            psum_s = stk.enter_context(tc.tile_pool(name=f"{tagp}psum_s",
                                                    bufs=3, space="PSUM"))
            psum_o = stk.enter_context(tc.tile_pool(name=f"{tagp}psum_o",
                                                    bufs=2, space="PSUM"))
            psum_d = stk.enter_context(tc.tile_pool(name=f"{tagp}psum_d",
                                                    bufs=1, space="PSUM"))
            expp = stk.enter_context(tc.tile_pool(name=f"{tagp}expp", bufs=2))
            maskp = stk.enter_context(tc.tile_pool(name=f"{tagp}maskp",
                                                   bufs=2))
            lnp = stk.enter_context(tc.tile_pool(name=f"{tagp}lnp", bufs=4))
            for c in range(NCH):
                qsl = slice(c * 512, (c + 1) * 512)
                vis = [t for t in range(KTI)
                       if not masked or _self_visible(t, c)]
                e8 = expp.tile([P, KTI, 512], f8, name="e8", tag="e8")
                for t in vis:
                    ps = psum_s.tile([P, 512], f32, name="ps_s", tag="ps_s")
                    mm_terms(ps, [(kT8, qT8, slice(t * P, (t + 1) * P))],
                             DPI, qsl, None)
                    if masked and _self_needs_mask(t, c):
                        m = maskp.tile([P, 512], f32, name="m", tag="mask")
                        nc.vector.tensor_scalar(out=m, in0=qgb[:, qsl],
                                                scalar1=kidx[:, t:t + 1],
                                                scalar2=-1e6, op0=ALU.is_lt,
                                                op1=ALU.mult)
                        nc.vector.tensor_add(out=ps, in0=ps, in1=m)
                    nc.scalar.activation(out=e8[:, t, :], in_=ps,
                                         func=ACT.Exp, bias=ln8b,
                                         scale=DS_EXP)
                npair = len(vis) // 2
                # denominators for the whole chunk: [1,512] = 32*ones^T @ E
                pd = psum_d.tile([1, 512], f32, name="pd", tag="pd")
                for r in range(npair):
                    nc.tensor.matmul(pd, lhsT=ones8[:, :, 0:1],
                                     rhs=e8[:, 2 * r:2 * r + 2, :],
                                     start=(r == 0), stop=(r == npair - 1),
                                     perf_mode=DR)
                dsb = lnp.tile([1, 512], f32, name="dsb", tag="dsb")
                nc.scalar.copy(out=dsb, in_=pd)
                dscr = dramp.tile([512], f32, name="dscr",
                                  tag=f"{tagp}dscr{c}")
                nc.sync.dma_start(out=dscr, in_=dsb)
                dT = lnp.tile([P, 4], f32, name="dT", tag="dT")
                nc.sync.dma_start(
                    out=dT, in_=dscr.rearrange("(a p) -> p a", p=P))
                recT = lnp.tile([P, 4], f32, name="recT", tag="recT")
                nc.vector.reciprocal(recT, dT)
                for u4 in range(4):
                    u = c * 4 + u4
                    # causal: local q-tile u sees k-pairs r <= u
                    rvis = range(u + 1) if masked else range(npair)
                    lsl = slice(u4 * P, (u4 + 1) * P)
                    po = psum_o.tile([P, 1024], f32, name="po", tag="po")
                    for n in range(2):
                        nsl = slice(n * 512, (n + 1) * 512)
                        last = 2 * len(rvis) - 1
                        k = 0
                        for rt in (v8, dv8):
                            for r in rvis:
                                nc.tensor.matmul(
                                    po[:, nsl],
                                    lhsT=e8[:, 2 * r:2 * r + 2, lsl],
                                    rhs=rt[:, 2 * r:2 * r + 2, nsl],
                                    start=(k == 0), stop=(k == last),
                                    perf_mode=DR)
                                k += 1
                    xr = lnp.tile([P, D], f16, name="xr", tag="xr", bufs=2)
                    nc.scalar.activation(out=xr, in_=po, func=ACT.Copy,
                                         scale=recT[:, u4:u4 + 1])
                    nc.vector.tensor_add(out=xr, in0=xr, in1=resid[u])
                    layer_norm(lnp, xr, gb, bb, y_out[u])

        def transpose_qd(stk, y_h, yT8, dyT8):
            # y_h[u]: [128q, 1024d] f16 -> yT8[:, i, q] (+ residual dyT8)
            psum_t = stk.enter_context(tc.tile_pool(name="psum_t", bufs=4,
                                                    space="PSUM"))
            tsp = stk.enter_context(tc.tile_pool(name="tsp", bufs=3))
            for c in range(NCH):
                for i in range(DTI):
                    for u4 in range(4):
                        u = c * 4 + u4
                        pt = psum_t.tile([P, P], f16, name="pt", tag="pt")
                        nc.tensor.transpose(
                            pt, in_=y_h[u][:, i * P:(i + 1) * P],
                            identity=ident)
                        osl = slice(c * 512 + u4 * P, c * 512 + (u4 + 1) * P)
                        if dyT8 is None:
                            nc.scalar.activation(out=yT8[:, i, osl], in_=pt,
                                                 func=ACT.Copy, scale=16.0)
                        else:
                            t16 = tsp.tile([P, P], f16, name="t16",
                                           tag="t16")
                            nc.scalar.activation(out=t16, in_=pt,
                                                 func=ACT.Copy, scale=16.0)
                            nc.vector.tensor_copy(out=yT8[:, i, osl],
                                                  in_=t16)
                            nc.vector.tensor_sub(out=dyT8[:, i, osl],
                                                 in0=t16, in1=yT8[:, i, osl])

        def emit_pass(pfx):
            # ------------ pools with cross-stage lifetimes ------------
            qkvp = tc.alloc_tile_pool(name=f"{pfx}qkvp", bufs=1)
            y1p = tc.alloc_tile_pool(name=f"{pfx}y1p", bufs=1, side="right")
            y1h = [y1p.tile([P, D], f16, name=f"y1h{u}", tag=f"y1h{u}")
                   for u in range(QTI)]

            # ===== stage A: self-attn projections (local full KV) =====
            kT8 = qkvp.tile([P, DTI, S], f8, name="kT8", tag="kT8")
            qT8 = qkvp.tile([P, DTI, NQ], f8, name="qT8", tag="qT8")
            v8 = qkvp.tile([P, KTI, D], f8, name="v8", tag="v8")
            dv8 = qkvp.tile([P, KTI, D], f8, name="dv8", tag="dv8")
            with ExitStack() as stA:
                kvp = stA.enter_context(tc.tile_pool(name=f"{pfx}kvp",
                                                     bufs=1))
                wp = stA.enter_context(tc.tile_pool(name=f"{pfx}wp", bufs=1))
                stgp = stA.enter_context(tc.tile_pool(name=f"{pfx}stgp",
                                                      bufs=3))
                psum_a = stA.enter_context(tc.tile_pool(name=f"{pfx}psum_a",
                                                        bufs=4, space="PSUM"))
                ykv8 = load_pairs(kvp, ykvT_d, "ykv8", D, S, tag="ykv")
                dykv8 = (load_pairs(kvp, dykvT_d, "dykv8", D, S, tag="dykv")
                         if XC_Y1 else None)
                wk = load_pairs(wp, w_d["wk1"], "wk1", D, D, tag="wA")
                dwk = load_pairs(wp, w_d["dwk1"], "dwk1", D, D, tag="dwA")
                project_dT(psum_a, qkvp, kT8, wk, dwk, ykv8, dykv8, S, "k1")
                wq = load_pairs(wp, w_d["wq1"], "wq1", D, D, tag="wB")
                dwq = load_pairs(wp, w_d["dwq1"], "dwq1", D, D, tag="dwB")
                yq8 = load_pairs(kvp, yqT_d, "yq8", D, NQ, tag="yq")
                dyq8 = (load_pairs(kvp, dyqT_d, "dyq8", D, NQ, tag="dyq")
                        if XC_Y1 else None)
                project_dT(psum_a, qkvp, qT8, wq, dwq, yq8, dyq8, NQ, "q1")
                wv = load_pairs(wp, w_d["wv1"], "wv1", D, D, tag="wA")
                dwv = load_pairs(wp, w_d["dwv1"], "dwv1", D, D, tag="dwA")
                project_v(psum_a, stgp, v8, dv8, wv, dwv, ykv8, dykv8, "v1")

            # ===== stage B: self-attention + LN1 =====
            with ExitStack() as stB:
                resp = stB.enter_context(tc.tile_pool(name=f"{pfx}resp",
                                                      bufs=1))
                gbp = stB.enter_context(tc.tile_pool(name=f"{pfx}gbp1",
                                                     bufs=1))
                yres = [resp.tile([P, D], f16, name=f"yres{u}",
                                  tag=f"yres{u}") for u in range(QTI)]
                for u in range(QTI):
                    nc.sync.dma_start(out=yres[u],
                                      in_=yres_d.ap()[u * P:(u + 1) * P, :])
                g1b = be1b = None
                if not gb_trivial:
                    g1b = load_vec_bcast(gbp, "g1")
                    be1b = load_vec_bcast(gbp, "be1")
                attention(stB, f"{pfx}sa_", qT8, kT8, v8, dv8, yres, g1b,
                          be1b, y1h, masked=True)
            qkvp.release()

            # transpose y1 -> y1T8 (+dy1T8) for cross-attn Q projection
            y1Tp = tc.alloc_tile_pool(name=f"{pfx}y1Tp", bufs=1)
            y1T8 = y1Tp.tile([P, DTI, NQ], f8, name="y1T8", tag="y1T8")
            dy1T8 = (y1Tp.tile([P, DTI, NQ], f8, name="dy1T8", tag="dy1T8")
                     if XC_Y1T else None)
            with ExitStack() as stB2:
                transpose_qd(stB2, y1h, y1T8, dy1T8)

            # ===== stage C: cross-attention + LN2 =====
            qkv2p = tc.alloc_tile_pool(name=f"{pfx}qkv2p", bufs=1,
                                       side="right")
            kT28 = qkv2p.tile([P, DTI, S], f8, name="kT28", tag="kT28")
            qT28 = qkv2p.tile([P, DTI, NQ], f8, name="qT28", tag="qT28")
            v28 = qkv2p.tile([P, KTI, D], f8, name="v28", tag="v28")
            dv28 = qkv2p.tile([P, KTI, D], f8, name="dv28", tag="dv28")
            with ExitStack() as stC1:
                zp = stC1.enter_context(tc.tile_pool(name=f"{pfx}zp", bufs=1))
                wp2 = stC1.enter_context(tc.tile_pool(name=f"{pfx}wp2",
                                                      bufs=1))
                stgp2 = stC1.enter_context(tc.tile_pool(name=f"{pfx}stgp2",
                                                        bufs=3))
                psum_c = stC1.enter_context(tc.tile_pool(name=f"{pfx}psum_c",
                                                         bufs=4,
                                                         space="PSUM"))
                z8 = load_pairs(zp, zT_d, "z8", D, S, tag="z8")
                dz8 = (load_pairs(zp, dzT_d, "dz8", D, S, tag="dz8")
                       if XC_Z else None)
                wk2 = load_pairs(wp2, w_d["wk2"], "wk2", D, D, tag="wC")
                dwk2 = load_pairs(wp2, w_d["dwk2"], "dwk2", D, D, tag="dwC")
                project_dT(psum_c, qkv2p, kT28, wk2, dwk2, z8, dz8, S, "k2")
                wq2 = load_pairs(wp2, w_d["wq2"], "wq2", D, D, tag="wD")
                dwq2 = load_pairs(wp2, w_d["dwq2"], "dwq2", D, D, tag="dwD")
                project_dT(psum_c, qkv2p, qT28, wq2, dwq2, y1T8, dy1T8, NQ,
                           "q2")
                wv2 = load_pairs(wp2, w_d["wv2"], "wv2", D, D, tag="wC")
                dwv2 = load_pairs(wp2, w_d["dwv2"], "dwv2", D, D, tag="dwC")
                project_v(psum_c, stgp2, v28, dv28, wv2, dwv2, z8, dz8, "v2")
            y1Tp.release()

            y2p = tc.alloc_tile_pool(name=f"{pfx}y2p", bufs=1)
            y2h = [y2p.tile([P, D], f16, name=f"y2h{u}", tag=f"y2h{u}")
                   for u in range(QTI)]
            # prefetch FFN2 weights under cross-attention PE work
            wf2p = tc.alloc_tile_pool(name=f"{pfx}wf2p", bufs=1)
            wf2 = load_pairs(wf2p, wf2_d, "wf2", DFF, D, tag="wf2")
            dwf2 = load_pairs(wf2p, dwf2_d, "dwf2", DFF, D, tag="dwf2")
            with ExitStack() as stC2:
                gbp2 = stC2.enter_context(tc.tile_pool(name=f"{pfx}gbp2",
                                                       bufs=1))
                g2b = be2b = None
                if not gb_trivial:
                    g2b = load_vec_bcast(gbp2, "g2")
                    be2b = load_vec_bcast(gbp2, "be2")
                attention(stC2, f"{pfx}ca_", qT28, kT28, v28, dv28, y1h, g2b,
                          be2b, y2h, masked=False)
            qkv2p.release()
            y1p.release()

            y2Tp = tc.alloc_tile_pool(name=f"{pfx}y2Tp", bufs=1)
            y2T8 = y2Tp.tile([P, DTI, NQ], f8, name="y2T8", tag="y2T8")
            dy2T8 = (y2Tp.tile([P, DTI, NQ], f8, name="dy2T8", tag="dy2T8")
                     if XC_Y2T else None)
            with ExitStack() as stC3:
                transpose_qd(stC3, y2h, y2T8, dy2T8)

            # ===== stage D: FFN + LN3 + output =====
            with ExitStack() as stD:
                wf1p = stD.enter_context(tc.tile_pool(name=f"{pfx}wf1p",
                                                      bufs=3))
                htp = stD.enter_context(tc.tile_pool(name=f"{pfx}htp",
                                                     bufs=2))
                gbp3 = stD.enter_context(tc.tile_pool(name=f"{pfx}gbp3",
                                                      bufs=1))
                outp = stD.enter_context(tc.tile_pool(name=f"{pfx}outp",
                                                      bufs=2))
                ln3p = stD.enter_context(tc.tile_pool(name=f"{pfx}ln3p",
                                                      bufs=4))
                psum_h = stD.enter_context(tc.tile_pool(name=f"{pfx}psum_h",
                                                        bufs=2, space="PSUM"))
                psum_f = stD.enter_context(tc.tile_pool(name=f"{pfx}psum_f",
                                                        bufs=2, space="PSUM"))
                g3b = be3b = bf2b = None
                if not gb_trivial:
                    g3b = load_vec_bcast(gbp3, "g3")
                    be3b = load_vec_bcast(gbp3, "be3")
                    bf2b = load_vec_bcast(gbp3, "bf2")

                for c in range(NCH):
                    csl = slice(c * 512, (c + 1) * 512)
                    h8 = htp.tile([P, FTI, 512], f8, name="h8", tag="h8")
                    for sp in range(FTI // 2):
                        ph = psum_h.tile([P, 1024], f32, name="ph", tag="ph")
                        for sh in range(2):
                            s = 2 * sp + sh
                            wt = wf1p.tile([P, DTI, P], f8, name="wf1s",
                                           tag=f"wf1s{sh}")
                            nc.sync.dma_start(
                                out=wt,
                                in_=wf1_d.ap()[:, s * P:(s + 1) * P]
                                .rearrange("(t p) m -> p t m", p=P))
                            dwt = wf1p.tile([P, DTI, P], f8, name="dwf1s",
                                            tag=f"dwf1s{sh}")
                            nc.sync.dma_start(
                                out=dwt,
                                in_=dwf1_d.ap()[:, s * P:(s + 1) * P]
                                .rearrange("(t p) m -> p t m", p=P))
                            fterms = [(wt, y2T8, slice(0, P)),
                                      (dwt, y2T8, slice(0, P))]
                            if dy2T8 is not None:
                                fterms.append((wt, dy2T8, slice(0, P)))
                            mm_terms(ph[:, sh * 512:(sh + 1) * 512], fterms,
                                     DPI, csl, None)
                            if not bf1_zero:
                                nc.scalar.activation(
                                    out=h8[:, s, :],
                                    in_=ph[:, sh * 512:(sh + 1) * 512],
                                    func=ACT.Relu,
                                    bias=bf1_sb[:, s:s + 1], scale=DS_H)
                        if bf1_zero:
                            nc.scalar.activation(
                                out=h8[:, 2 * sp:2 * sp + 2, :], in_=ph,
                                func=ACT.Relu, bias=0.0, scale=DS_H)
                    for u4 in range(4):
                        u = c * 4 + u4
                        lsl = slice(u4 * P, (u4 + 1) * P)
                        pf = psum_f.tile([P, D], f32, name="pf", tag="pf")
                        for n in range(2):
                            nsl = slice(n * 512, (n + 1) * 512)
                            last = 2 * FPI - 1
                            k = 0
                            for rt in (wf2, dwf2):
                                for r in range(FPI):
                                    nc.tensor.matmul(
                                        pf[:, nsl],
                                        lhsT=h8[:, 2 * r:2 * r + 2, lsl],
                                        rhs=rt[:, 2 * r:2 * r + 2, nsl],
                                        start=(k == 0), stop=(k == last),
                                        perf_mode=DR)
                                    k += 1
                        xr = ln3p.tile([P, D], f16, name="xr3", tag="xr3",
                                       bufs=2)
                        nc.vector.tensor_scalar(out=xr, in0=pf, scalar1=DS_F,
                                                scalar2=None, op0=ALU.mult)
                        if not gb_trivial:
                            nc.vector.tensor_add(out=xr, in0=xr, in1=bf2b)
                        nc.vector.tensor_add(out=xr, in0=xr, in1=y2h[u])
                        y3 = outp.tile([P, D], f32, name="y3", tag="y3")
                        layer_norm(ln3p, xr, g3b, be3b, y3)
                        nc.sync.dma_start(
                            out=out_d.ap()[u * P:(u + 1) * P, :], in_=y3)
            y2Tp.release()
            wf2p.release()
            y2p.release()

        emit_pass("")

    nc.compile()
    return nc


_CACHE = {}


def _get_nc(gb_trivial=True, bf1_zero=True):
    key = (gb_trivial, bf1_zero, XC_Y1, XC_Z, XC_Y1T, XC_Y2T)
    if key not in _CACHE:
        _CACHE[key] = build_nc(gb_trivial=gb_trivial, bf1_zero=bf1_zero)
    return _CACHE[key]


def _q_indices(h):
    """Interleaved q-tile ownership: core-half h owns global tiles
    h, h+2, ..."""
    tiles = np.arange(h, 2 * QTI, 2)
    return (tiles[:, None] * P + np.arange(P)[None, :]).reshape(-1)


def _q8(x):
    return np.ascontiguousarray(x).astype(E4)


def _q8r(x):
    """fp8 value + same-scale residual."""
    hi = _q8(x)
    lo = _q8(x - hi.astype(np.float32))
    return hi, lo


def _prep_core(c, y, Z, shared):
    b, h = c // 2, c % 2
    qi = _q_indices(h)
    yb = y[b]
    yqT, dyqT = _q8r(16.0 * yb[qi].T)
    ykvT, dykvT = _q8r(16.0 * yb.T)
    zT, dzT = _q8r(16.0 * Z[b].T)
    m = {
        "yqT": yqT, "dyqT": dyqT,
        "ykvT": ykvT, "dykvT": dykvT,
        "zT": zT, "dzT": dzT,
        "yres": yb[qi].astype(np.float16),
        "qg": qi.astype(np.float32),
        "kg": np.arange(S, dtype=np.float32),
    }
    m.update(shared)
    return m


def kernel(**inputs):
    inp = {k: np.asarray(v) for k, v in inputs.items()}
    y = inp["y"].astype(np.float32)
    Z = inp["Z"].astype(np.float32)
    shared = {"ones8": np.full((2 * P, 16), 32.0, np.float32).astype(E4),
              "bf1": np.ascontiguousarray(
                  16.0 * inp["b_ff1"].astype(np.float32)
                  .reshape(FTI, P).T)}
    for name, k in [("wq1", "WQ1"), ("wk1", "WK1"), ("wv1", "WV1"),
                    ("wq2", "WQ2"), ("wk2", "WK2"), ("wv2", "WV2"),
                    ("wf1", "W_ff1"), ("wf2", "W_ff2")]:
        hi, lo = _q8r(1024.0 * inp[k].astype(np.float32))
        shared[name] = hi
        shared["d" + name] = lo
    for name, k in [("bf2", "b_ff2"), ("g1", "g1"), ("be1", "be1"),
                    ("g2", "g2"), ("be2", "be2"), ("g3", "g3"),
                    ("be3", "be3")]:
        shared[name] = inp[k].astype(np.float32)
    gb_trivial = bool(
        np.all(inp["g1"] == 1) and np.all(inp["g2"] == 1)
        and np.all(inp["g3"] == 1) and np.all(inp["be1"] == 0)
        and np.all(inp["be2"] == 0) and np.all(inp["be3"] == 0)
        and np.all(inp["b_ff2"] == 0))
    bf1_zero = bool(np.all(inp["b_ff1"] == 0))
    in_maps = [_prep_core(c, y, Z, shared) for c in range(N_CORES)]
    res = run_bass_kernel_spmd(_get_nc(gb_trivial, bf1_zero), in_maps,
                               list(range(N_CORES)))
    out = np.zeros((4, 2048, 1024), np.float32)
    for c in range(N_CORES):
        b, h = c // 2, c % 2
        out[b, _q_indices(h)] = res.results[c]["out"]
    return out
